# revision 18
# baseline (speedup 1.0000x reference)
"""Bass/Trainium2 kernel for nn_BayesianResNet_71408126263673.

Grouped per-sample conv: for each of 32 samples i,
  out[i] = conv2d(x[i] [128,32,32], W[i] [128oc,128c,3,3], pad=1, stride=1) + bias[i]

Sharding: b_i (32 samples) split across 8 NeuronCores, 4 samples per core.
Pure data parallel, no collectives.

Per-core kernel: each sample's conv is computed as 9 accumulating matmuls
(one per 3x3 tap) into PSUM:
  out[oc, pix] = sum_{kh,kw} W[:, :, kh, kw].T @ xpad[:, shifted pix]
with K=c=128 (partition/contraction), M=oc=128, N=512 pixels (16 output rows
per PSUM bank). The input image is zero-padded to 34x34 on the HOST so DMA
loads are fully contiguous and no memset/masking is needed on-chip. Weights
are pre-transposed on the host to [c, kh*kw, oc] so each tap is a ready-to-use
lhsT (stationary operand) tile.
"""

import os
import numpy as np

import concourse.bacc as bacc
import concourse.tile as tile
from concourse import mybir
from concourse.bass_utils import run_bass_kernel_spmd

N_CORES = 8
B_I, B_J, C, H, W = 32, 1, 128, 32, 32
OC, KH, KW = 128, 3, 3
S = B_I // N_CORES            # samples per core
HP, WP = H + 2, W + 2         # padded image
NTAP = KH * KW                # 9
NBLK = 2                      # output row blocks per sample
RPB = H // NBLK               # 16 rows per block -> N = 512

_MM_DT_NAME = os.environ.get("CONV_MM_DTYPE", "fp32r")
MM_DT = {
    "fp32": mybir.dt.float32,
    "fp32r": mybir.dt.float32r,
    "fp16": mybir.dt.float16,
    "bf16": mybir.dt.bfloat16,
}[_MM_DT_NAME]
MM_NP = {
    "fp32": np.float32,
    "fp32r": np.float32,
    "fp16": np.float16,
    "bf16": None,  # ml_dtypes.bfloat16, filled lazily
}[_MM_DT_NAME]
if _MM_DT_NAME == "bf16":
    import ml_dtypes

    MM_NP = ml_dtypes.bfloat16

# test.py hooks: set TRACE=True before calling kernel() to profile; the
# BassKernelResults of the last run lands in LAST_RESULTS.
TRACE = False
TRACE_KW = {}
LAST_RESULTS = None

_NC_CACHE = None


def _build_nc():
    f32 = mybir.dt.float32
    nc = bacc.Bacc()
    x_d = nc.declare_dram_parameter("x", [S, C, HP, WP], MM_DT, isOutput=False)
    w_d = nc.declare_dram_parameter("w", [S, C, NTAP * OC], MM_DT, isOutput=False)
    b_d = nc.declare_dram_parameter("b", [OC, S], f32, isOutput=False)
    o_d = nc.declare_dram_parameter("o", [S, OC, H, W], f32, isOutput=True)

    with tile.TileContext(nc) as tc:
        with (
            tc.tile_pool(name="ins", bufs=1) as ins_pool,
            tc.tile_pool(name="outs", bufs=1) as outs_pool,
            tc.tile_pool(name="psum", bufs=8, space="PSUM") as psum_pool,
        ):
            # x loads on the SP HWDGE queue, w loads on the ACT HWDGE queue so
            # sample 0's image and weights stream in parallel; the tiny bias
            # load trails the weight queue (first needed ~10us later).
            xps, wts = [], []
            for s in range(S):
                xp = ins_pool.tile([C, HP, WP], MM_DT, tag=f"xpad{s}")
                wt = ins_pool.tile([C, NTAP * OC], MM_DT, tag=f"w{s}")
                if s == 0:
                    # Small first chunks prime both DGE queues and unblock the
                    # first matmuls (row-block 0 consumes tap t at ~220ns/tap,
                    # so taps 1-8 can land while tap 0's matmul runs).
                    nc.sync.dma_start(xp[:, : RPB + 2, :], x_d[s][:, : RPB + 2, :])
                    nc.scalar.dma_start(wt[:, :OC], w_d[s][:, :OC])
                    nc.sync.dma_start(xp[:, RPB + 2 :, :], x_d[s][:, RPB + 2 :, :])
                    nc.scalar.dma_start(wt[:, OC:], w_d[s][:, OC:])
                else:
                    nc.sync.dma_start(xp[:], x_d[s])
                    nc.scalar.dma_start(wt[:], w_d[s])
                xps.append(xp)
                wts.append(wt)

            bias_t = ins_pool.tile([OC, S], f32, tag="bias")
            nc.scalar.dma_start(bias_t[:], b_d[:])

            for s in range(S):
                out_t = outs_pool.tile([OC, H, W], f32, tag=f"out{s}")
                for rb in range(NBLK):
                    ps = psum_pool.tile([OC, RPB, W], f32)
                    for t in range(NTAP):
                        kh, kw = divmod(t, KW)
                        rhs = xps[s][:, rb * RPB + kh : rb * RPB + kh + RPB,
                                     kw : kw + W]
                        lhsT = wts[s][:, t * OC : (t + 1) * OC]
                        nc.tensor.matmul(
                            ps[:],
                            lhsT,
                            rhs,
                            start=(t == 0),
                            stop=(t == NTAP - 1),
                        )
                    nc.scalar.activation(
                        out_t[:, rb * RPB : (rb + 1) * RPB, :],
                        ps[:],
                        mybir.ActivationFunctionType.Identity,
                        bias=bias_t[:, s : s + 1],
                    )
                nc.sync.dma_start(o_d[s], out_t[:])
    nc.compile()
    return nc


def _get_nc():
    global _NC_CACHE
    if _NC_CACHE is None:
        _NC_CACHE = _build_nc()
    return _NC_CACHE


def kernel(x: np.ndarray, weight: np.ndarray, bias: np.ndarray) -> np.ndarray:
    global LAST_RESULTS
    assert x.shape == (B_I, B_J, C, H, W)
    assert weight.shape == (B_I, OC, C, KH, KW)
    assert bias.shape == (B_I, B_J, OC)

    x = np.asarray(x, dtype=np.float32)
    weight = np.asarray(weight, dtype=np.float32)
    bias = np.asarray(bias, dtype=np.float32)

    # Host-side layout prep (part of sharding): zero-pad images, transpose
    # weights so each 3x3 tap is a contiguous [c, oc] stationary tile.
    xp = np.zeros((B_I, C, HP, WP), dtype=MM_NP)
    xp[:, :, 1 : 1 + H, 1 : 1 + W] = x[:, 0].astype(MM_NP)
    wt = np.ascontiguousarray(weight.transpose(0, 2, 3, 4, 1))  # [b_i, c, kh, kw, oc]
    wt = wt.reshape(B_I, C, NTAP * OC).astype(MM_NP)
    bt = bias[:, 0, :]  # [b_i, oc]

    in_maps = []
    for core in range(N_CORES):
        sl = slice(core * S, (core + 1) * S)
        in_maps.append(
            {
                "x": np.ascontiguousarray(xp[sl]),
                "w": np.ascontiguousarray(wt[sl]),
                "b": np.ascontiguousarray(bt[sl].T),  # [OC, S]
            }
        )

    nc = _get_nc()
    res = run_bass_kernel_spmd(
        nc, in_maps, core_ids=list(range(N_CORES)), trace=TRACE, **TRACE_KW
    )
    LAST_RESULTS = res

    out = np.concatenate([res.results[c]["o"] for c in range(N_CORES)], axis=0)
    return out.reshape(B_I, B_J, OC, H, W)


# revision 20
# speedup vs baseline: 1.0046x; 1.0046x over previous
"""Bass/Trainium2 kernel for nn_BayesianResNet_71408126263673.

Grouped per-sample conv: for each of 32 samples i,
  out[i] = conv2d(x[i] [128,32,32], W[i] [128oc,128c,3,3], pad=1, stride=1) + bias[i]

Sharding: b_i (32 samples) split across 8 NeuronCores, 4 samples per core.
Pure data parallel, no collectives.

Per-core kernel: each sample's conv is computed as 9 accumulating matmuls
(one per 3x3 tap) into PSUM:
  out[oc, pix] = sum_{kh,kw} W[:, :, kh, kw].T @ xpad[:, shifted pix]
with K=c=128 (partition/contraction), M=oc=128, N=512 pixels (16 output rows
per PSUM bank). The input image is zero-padded to 34x34 on the HOST so DMA
loads are fully contiguous and no memset/masking is needed on-chip. Weights
are pre-transposed on the host to [c, kh*kw, oc] so each tap is a ready-to-use
lhsT (stationary operand) tile.
"""

import os
import numpy as np

import concourse.bacc as bacc
import concourse.tile as tile
from concourse import mybir
from concourse.bass_utils import run_bass_kernel_spmd

N_CORES = 8
B_I, B_J, C, H, W = 32, 1, 128, 32, 32
OC, KH, KW = 128, 3, 3
S = B_I // N_CORES            # samples per core
HP, WP = H + 2, W + 2         # padded image
NTAP = KH * KW                # 9
NBLK = 2                      # output row blocks per sample
RPB = H // NBLK               # 16 rows per block -> N = 512

_MM_DT_NAME = os.environ.get("CONV_MM_DTYPE", "fp32r")
MM_DT = {
    "fp32": mybir.dt.float32,
    "fp32r": mybir.dt.float32r,
    "fp16": mybir.dt.float16,
    "bf16": mybir.dt.bfloat16,
}[_MM_DT_NAME]
MM_NP = {
    "fp32": np.float32,
    "fp32r": np.float32,
    "fp16": np.float16,
    "bf16": None,  # ml_dtypes.bfloat16, filled lazily
}[_MM_DT_NAME]
if _MM_DT_NAME == "bf16":
    import ml_dtypes

    MM_NP = ml_dtypes.bfloat16

# test.py hooks: set TRACE=True before calling kernel() to profile; the
# BassKernelResults of the last run lands in LAST_RESULTS.
TRACE = False
TRACE_KW = {}
LAST_RESULTS = None

_NC_CACHE = None


def _build_nc():
    f32 = mybir.dt.float32
    nc = bacc.Bacc()
    x_d = nc.declare_dram_parameter("x", [S, C, HP, WP], MM_DT, isOutput=False)
    w_d = nc.declare_dram_parameter("w", [S, C, NTAP * OC], MM_DT, isOutput=False)
    b_d = nc.declare_dram_parameter("b", [OC, S], f32, isOutput=False)
    o_d = nc.declare_dram_parameter("o", [S, OC, H, W], f32, isOutput=True)

    with tile.TileContext(nc) as tc:
        with (
            tc.tile_pool(name="ins", bufs=1) as ins_pool,
            tc.tile_pool(name="outs", bufs=1) as outs_pool,
            tc.tile_pool(name="psum", bufs=8, space="PSUM") as psum_pool,
        ):
            # The SP HWDGE queue ramps to full rate immediately while the ACT
            # queue starts slow, so sample 0's critical operands (w0, then the
            # first row-block of x0) go on SP in deadline order. Later
            # samples' weights + the bias ride the ACT queue, whose slow ramp
            # doesn't matter (first needed ~5us later).
            xps = [
                ins_pool.tile([C, HP, WP], MM_DT, tag=f"xpad{s}", name=f"xpad{s}")
                for s in range(S)
            ]
            wts = [
                ins_pool.tile([C, NTAP * OC], MM_DT, tag=f"w{s}", name=f"w{s}")
                for s in range(S)
            ]
            bias_t = ins_pool.tile([OC, S], f32, tag="bias")

            nc.sync.dma_start(wts[0][:], w_d[0])
            nc.sync.dma_start(xps[0][:, : RPB + 2, :], x_d[0][:, : RPB + 2, :])
            nc.scalar.dma_start(bias_t[:], b_d[:])
            nc.scalar.dma_start(wts[1][:], w_d[1])
            nc.sync.dma_start(xps[0][:, RPB + 2 :, :], x_d[0][:, RPB + 2 :, :])
            nc.sync.dma_start(xps[1][:], x_d[1])
            nc.scalar.dma_start(xps[2][:], x_d[2])
            nc.scalar.dma_start(wts[2][:], w_d[2])
            nc.sync.dma_start(xps[3][:], x_d[3])
            nc.scalar.dma_start(wts[3][:], w_d[3])

            for s in range(S):
                out_t = outs_pool.tile([OC, H, W], f32, tag=f"out{s}")
                for rb in range(NBLK):
                    ps = psum_pool.tile([OC, RPB, W], f32)
                    for t in range(NTAP):
                        kh, kw = divmod(t, KW)
                        rhs = xps[s][:, rb * RPB + kh : rb * RPB + kh + RPB,
                                     kw : kw + W]
                        lhsT = wts[s][:, t * OC : (t + 1) * OC]
                        nc.tensor.matmul(
                            ps[:],
                            lhsT,
                            rhs,
                            start=(t == 0),
                            stop=(t == NTAP - 1),
                        )
                    nc.scalar.activation(
                        out_t[:, rb * RPB : (rb + 1) * RPB, :],
                        ps[:],
                        mybir.ActivationFunctionType.Identity,
                        bias=bias_t[:, s : s + 1],
                    )
                nc.sync.dma_start(o_d[s], out_t[:])
    nc.compile()
    return nc


def _get_nc():
    global _NC_CACHE
    if _NC_CACHE is None:
        _NC_CACHE = _build_nc()
    return _NC_CACHE


def kernel(x: np.ndarray, weight: np.ndarray, bias: np.ndarray) -> np.ndarray:
    global LAST_RESULTS
    assert x.shape == (B_I, B_J, C, H, W)
    assert weight.shape == (B_I, OC, C, KH, KW)
    assert bias.shape == (B_I, B_J, OC)

    x = np.asarray(x, dtype=np.float32)
    weight = np.asarray(weight, dtype=np.float32)
    bias = np.asarray(bias, dtype=np.float32)

    # Host-side layout prep (part of sharding): zero-pad images, transpose
    # weights so each 3x3 tap is a contiguous [c, oc] stationary tile.
    xp = np.zeros((B_I, C, HP, WP), dtype=MM_NP)
    xp[:, :, 1 : 1 + H, 1 : 1 + W] = x[:, 0].astype(MM_NP)
    wt = np.ascontiguousarray(weight.transpose(0, 2, 3, 4, 1))  # [b_i, c, kh, kw, oc]
    wt = wt.reshape(B_I, C, NTAP * OC).astype(MM_NP)
    bt = bias[:, 0, :]  # [b_i, oc]

    in_maps = []
    for core in range(N_CORES):
        sl = slice(core * S, (core + 1) * S)
        in_maps.append(
            {
                "x": np.ascontiguousarray(xp[sl]),
                "w": np.ascontiguousarray(wt[sl]),
                "b": np.ascontiguousarray(bt[sl].T),  # [OC, S]
            }
        )

    nc = _get_nc()
    res = run_bass_kernel_spmd(
        nc, in_maps, core_ids=list(range(N_CORES)), trace=TRACE, **TRACE_KW
    )
    LAST_RESULTS = res

    out = np.concatenate([res.results[c]["o"] for c in range(N_CORES)], axis=0)
    return out.reshape(B_I, B_J, OC, H, W)


# revision 25
# speedup vs baseline: 1.0429x; 1.0381x over previous
"""Bass/Trainium2 kernel for nn_BayesianResNet_71408126263673.

Grouped per-sample conv: for each of 32 samples i,
  out[i] = conv2d(x[i] [128,32,32], W[i] [128oc,128c,3,3], pad=1, stride=1) + bias[i]

Sharding: b_i (32 samples) split across 8 NeuronCores, 4 samples per core.
Pure data parallel, no collectives.

Per-core kernel: each sample's conv is computed as 9 accumulating matmuls
(one per 3x3 tap) into PSUM:
  out[oc, pix] = sum_{kh,kw} W[:, :, kh, kw].T @ xpad[:, shifted pix]
with K=c=128 (partition/contraction), M=oc=128, N=512 pixels (16 output rows
per PSUM bank). The input image is zero-padded to 34x34 on the HOST so DMA
loads are fully contiguous and no memset/masking is needed on-chip. Weights
are pre-transposed on the host to [c, kh*kw, oc] so each tap is a ready-to-use
lhsT (stationary operand) tile.
"""

import os
import numpy as np

import concourse.bacc as bacc
import concourse.tile as tile
from concourse import mybir
from concourse.bass_utils import run_bass_kernel_spmd

N_CORES = 8
B_I, B_J, C, H, W = 32, 1, 128, 32, 32
OC, KH, KW = 128, 3, 3
S = B_I // N_CORES            # samples per core
HP, WP = H + 2, W + 2         # padded image
NTAP = KH * KW                # 9
NBLK = 2                      # output row blocks per sample
RPB = H // NBLK               # 16 rows per block -> N = 512

_MM_DT_NAME = os.environ.get("CONV_MM_DTYPE", "fp32r")
MM_DT = {
    "fp32": mybir.dt.float32,
    "fp32r": mybir.dt.float32r,
    "fp16": mybir.dt.float16,
    "bf16": mybir.dt.bfloat16,
}[_MM_DT_NAME]
MM_NP = {
    "fp32": np.float32,
    "fp32r": np.float32,
    "fp16": np.float16,
    "bf16": None,  # ml_dtypes.bfloat16, filled lazily
}[_MM_DT_NAME]
if _MM_DT_NAME == "bf16":
    import ml_dtypes

    MM_NP = ml_dtypes.bfloat16

# test.py hooks: set TRACE=True before calling kernel() to profile; the
# BassKernelResults of the last run lands in LAST_RESULTS.
TRACE = False
TRACE_KW = {}
LAST_RESULTS = None

_NC_CACHE = None


def _build_nc():
    f32 = mybir.dt.float32
    nc = bacc.Bacc()
    x_d = nc.declare_dram_parameter("x", [S, C, HP, WP], MM_DT, isOutput=False)
    w_d = nc.declare_dram_parameter("w", [S, C, NTAP * OC], MM_DT, isOutput=False)
    b_d = nc.declare_dram_parameter("b", [OC, S], f32, isOutput=False)
    o_d = nc.declare_dram_parameter("o", [S, OC, H, W], f32, isOutput=True)

    with tile.TileContext(nc) as tc:
        with (
            tc.tile_pool(name="ins", bufs=1) as ins_pool,
            tc.tile_pool(name="outs", bufs=1) as outs_pool,
            tc.tile_pool(name="psum", bufs=8, space="PSUM") as psum_pool,
        ):
            # The SP HWDGE queue ramps to full rate immediately while the ACT
            # queue starts slow, so sample 0's critical operands (w0, then the
            # first row-block of x0) go on SP in deadline order. Later
            # samples' weights + the bias ride the ACT queue, whose slow ramp
            # doesn't matter (first needed ~5us later).
            # PE warmup: ~32 dependency-free matmuls on garbage data keep the
            # PE busy from engine start so the HAM clock-gate reaches 2.4 GHz
            # before the first real matmul (otherwise the first ~3.4us of
            # matmuls run at 1.2 GHz). Their PSUM tile is never read.
            wu_x = ins_pool.tile([C, OC], MM_DT, tag="warmup", name="warmup")
            nc.gpsimd.memset(wu_x[:], 0.0)
            wu_ps = psum_pool.tile([C, OC], f32, name="wu_ps", tag="ps")
            for _ in range(32):
                nc.tensor.matmul(wu_ps[:], wu_x[:], wu_x[:], start=True, stop=True)

            xps = [
                ins_pool.tile([C, HP, WP], MM_DT, tag=f"xpad{s}", name=f"xpad{s}")
                for s in range(1, S)
            ]
            # Sample 0's image is loaded as two overlapping row-block tiles so
            # the first row-block's matmuls aren't gated on the second chunk.
            xp0a = ins_pool.tile([C, RPB + 2, WP], MM_DT, tag="xp0a", name="xp0a")
            xp0b = ins_pool.tile([C, RPB + 2, WP], MM_DT, tag="xp0b", name="xp0b")
            xps = [(xp0a, xp0b)] + xps
            wts = [
                ins_pool.tile([C, NTAP * OC], MM_DT, tag=f"w{s}", name=f"w{s}")
                for s in range(S)
            ]
            bias_t = ins_pool.tile([OC, S], f32, tag="bias")

            nc.sync.dma_start(wts[0][:], w_d[0])
            nc.sync.dma_start(xp0a[:], x_d[0][:, : RPB + 2, :])
            nc.scalar.dma_start(bias_t[:], b_d[:])
            nc.scalar.dma_start(wts[1][:], w_d[1])
            nc.sync.dma_start(xp0b[:], x_d[0][:, RPB : RPB + RPB + 2, :])
            nc.sync.dma_start(xps[1][:], x_d[1])
            nc.scalar.dma_start(xps[2][:], x_d[2])
            nc.scalar.dma_start(wts[2][:], w_d[2])
            nc.sync.dma_start(xps[3][:], x_d[3])
            nc.scalar.dma_start(wts[3][:], w_d[3])

            for s in range(S):
                out_t = outs_pool.tile([OC, H, W], f32, tag=f"out{s}", name=f"out{s}")
                for rb in range(NBLK):
                    ps = psum_pool.tile([OC, RPB, W], f32, name=f"ps{s}_{rb}", tag="ps")
                    for t in range(NTAP):
                        kh, kw = divmod(t, KW)
                        if s == 0:
                            src = xps[0][rb]
                            rhs = src[:, kh : kh + RPB, kw : kw + W]
                        else:
                            rhs = xps[s][:, rb * RPB + kh : rb * RPB + kh + RPB,
                                         kw : kw + W]
                        lhsT = wts[s][:, t * OC : (t + 1) * OC]
                        nc.tensor.matmul(
                            ps[:],
                            lhsT,
                            rhs,
                            start=(t == 0),
                            stop=(t == NTAP - 1),
                        )
                    nc.scalar.activation(
                        out_t[:, rb * RPB : (rb + 1) * RPB, :],
                        ps[:],
                        mybir.ActivationFunctionType.Identity,
                        bias=bias_t[:, s : s + 1],
                    )
                    if s == S - 1:
                        # Split the last sample's store so only 256KB remains
                        # after the final ACT.
                        nc.sync.dma_start(
                            o_d[s][:, rb * RPB : (rb + 1) * RPB, :],
                            out_t[:, rb * RPB : (rb + 1) * RPB, :],
                        )
                if s < S - 1:
                    nc.sync.dma_start(o_d[s], out_t[:])
    nc.compile()
    return nc


def _get_nc():
    global _NC_CACHE
    if _NC_CACHE is None:
        _NC_CACHE = _build_nc()
    return _NC_CACHE


def kernel(x: np.ndarray, weight: np.ndarray, bias: np.ndarray) -> np.ndarray:
    global LAST_RESULTS
    assert x.shape == (B_I, B_J, C, H, W)
    assert weight.shape == (B_I, OC, C, KH, KW)
    assert bias.shape == (B_I, B_J, OC)

    x = np.asarray(x, dtype=np.float32)
    weight = np.asarray(weight, dtype=np.float32)
    bias = np.asarray(bias, dtype=np.float32)

    # Host-side layout prep (part of sharding): zero-pad images, transpose
    # weights so each 3x3 tap is a contiguous [c, oc] stationary tile.
    xp = np.zeros((B_I, C, HP, WP), dtype=MM_NP)
    xp[:, :, 1 : 1 + H, 1 : 1 + W] = x[:, 0].astype(MM_NP)
    wt = np.ascontiguousarray(weight.transpose(0, 2, 3, 4, 1))  # [b_i, c, kh, kw, oc]
    wt = wt.reshape(B_I, C, NTAP * OC).astype(MM_NP)
    bt = bias[:, 0, :]  # [b_i, oc]

    in_maps = []
    for core in range(N_CORES):
        sl = slice(core * S, (core + 1) * S)
        in_maps.append(
            {
                "x": np.ascontiguousarray(xp[sl]),
                "w": np.ascontiguousarray(wt[sl]),
                "b": np.ascontiguousarray(bt[sl].T),  # [OC, S]
            }
        )

    nc = _get_nc()
    res = run_bass_kernel_spmd(
        nc, in_maps, core_ids=list(range(N_CORES)), trace=TRACE, **TRACE_KW
    )
    LAST_RESULTS = res

    out = np.concatenate([res.results[c]["o"] for c in range(N_CORES)], axis=0)
    return out.reshape(B_I, B_J, OC, H, W)
